# revision 4
# baseline (speedup 1.0000x reference)
"""Trainium2 Bass kernel for the Diffusion get_energy problem.

Math (per graph b, all computed on one NeuronCore; data-parallel over the
8 graphs across 8 cores):

  rot = QR(pre_rot).Q                        (host, tiny)
  new_lig[t,l] = rot[t] @ lig_coord[l] + trans[t]          (host, tiny)
  atn[l,r,e]  = sum_f lig_feat[l,e,f]*rec_feat[r,e,f] * mask[l,r]   (PE)
  d2[t,l,r]   = |new_lig[t,l] - rec_coord[r]|^2            (PE, K=5 matmul)
  U[b,t] = sum_{l,r,e} atn[l,r,e] * d(t,l,r)^exps[e],  exps=[-3,-2,-1,1,2]

Channels -3..+1 go through an Ln/Exp power evaluation (ScalarE, single
activation-table set) + bf16 elementwise products (VectorE/GpSimd) + one-hot
reduction matmuls (PE).  Channel +2 (d^2) is separable and computed
analytically with small matmuls in fp32:
  sum a2*d2 = sum_l nlaug2[t,l,:] . W[l,:],  W = atn2^T @ [y,1,rec2]
"""

import numpy as np
import ml_dtypes

B, T, L, R, E, F = 8, 16, 128, 1024, 5, 512
KF = F // 128  # 4 f-blocks of 128
NCHIP = 8

_BUILT = None  # cached (nc, meta)


# --------------------------------------------------------------------------
# device program
# --------------------------------------------------------------------------
def build_nc():
    from contextlib import ExitStack

    import concourse.bacc as bacc
    import concourse.mybir as mybir
    import concourse.tile as tile

    f32 = mybir.dt.float32
    bf16 = mybir.dt.bfloat16
    AF = mybir.ActivationFunctionType
    MUL = mybir.AluOpType.mult

    nc = bacc.Bacc("TRN2", target_bir_lowering=False)

    d_ligT = nc.dram_tensor("ligT", [128, E * KF * L], f32, kind="ExternalInput")
    d_recT = nc.dram_tensor("recT", [128, E * KF * R], f32, kind="ExternalInput")
    d_nlaug = nc.dram_tensor("nlaug", [5, T * L], f32, kind="ExternalInput")
    d_recaug = nc.dram_tensor("recaug", [5, R], f32, kind="ExternalInput")
    d_nl2d = nc.dram_tensor("nl2d", [128, 5 * T], f32, kind="ExternalInput")
    d_ydev = nc.dram_tensor("ydev", [128, 8 * 5], f32, kind="ExternalInput")
    d_ligm = nc.dram_tensor("ligm", [1, L], f32, kind="ExternalInput")
    d_recm = nc.dram_tensor("recm", [1, R], f32, kind="ExternalInput")
    d_ident = nc.dram_tensor("ident", [128, 128], f32, kind="ExternalInput")
    d_onehot = nc.dram_tensor("onehot", [128, T * T], bf16, kind="ExternalInput")
    d_u4 = nc.dram_tensor("u4", [16, 1], f32, kind="ExternalOutput")
    d_u2 = nc.dram_tensor("u2", [1, 16], f32, kind="ExternalOutput")

    with ExitStack() as ctx:
        tc = ctx.enter_context(tile.TileContext(nc))
        const = ctx.enter_context(tc.tile_pool(name="const", bufs=1))
        recp = ctx.enter_context(tc.tile_pool(name="recp", bufs=2))
        work = ctx.enter_context(tc.tile_pool(name="work", bufs=3))
        psA = ctx.enter_context(tc.tile_pool(name="psA", bufs=2, space="PSUM"))
        psD = ctx.enter_context(tc.tile_pool(name="psD", bufs=2, space="PSUM"))
        psU = ctx.enter_context(tc.tile_pool(name="psU", bufs=1, space="PSUM"))
        psX = ctx.enter_context(tc.tile_pool(name="psX", bufs=1, space="PSUM"))

        # ---- constant loads ------------------------------------------------
        t_ligT = const.tile([128, E * KF * L], f32)
        nc.sync.dma_start(out=t_ligT[:], in_=d_ligT[:])
        t_nlaug = const.tile([5, T * L], f32)
        nc.sync.dma_start(out=t_nlaug[:], in_=d_nlaug[:])
        t_recaug = const.tile([5, R], f32)
        nc.sync.dma_start(out=t_recaug[:], in_=d_recaug[:])
        t_nl2d = const.tile([128, 5 * T], f32)
        nc.sync.dma_start(out=t_nl2d[:], in_=d_nl2d[:])
        t_ydev = const.tile([128, 8 * 5], f32)
        nc.sync.dma_start(out=t_ydev[:], in_=d_ydev[:])
        t_ligm = const.tile([1, L], f32)
        nc.sync.dma_start(out=t_ligm[:], in_=d_ligm[:])
        t_recm = const.tile([1, R], f32)
        nc.sync.dma_start(out=t_recm[:], in_=d_recm[:])
        t_ident = const.tile([128, 128], f32)
        nc.sync.dma_start(out=t_ident[:], in_=d_ident[:])
        t_onehot = const.tile([128, T * T], bf16)
        nc.sync.dma_start(out=t_onehot[:], in_=d_onehot[:])
        t_eps = const.tile([128, 1], f32)
        nc.vector.memset(t_eps[:], 1e-6)

        # ---- pair mask (outer product via K=1 matmul) ----------------------
        t_pm = const.tile([128, R], f32)
        for h in range(2):
            ps_pm = psX.tile([128, 512], f32, tag="aux")
            nc.tensor.matmul(
                ps_pm[:],
                lhsT=t_ligm[:],
                rhs=t_recm[:, h * 512 : (h + 1) * 512],
                start=True,
                stop=True,
            )
            nc.scalar.copy(out=t_pm[:, h * 512 : (h + 1) * 512], in_=ps_pm[:])

        # ---- atn coefficients ---------------------------------------------
        # channels 0..3 -> bf16 cat buffer (strip order matches exps order
        # [-3,-2,-1,+1]); channel 4 (d^2) -> fp32 for the analytic path.
        t_atncat = const.tile([128, 4 * R], bf16)
        t_atn2 = const.tile([128, R], f32)
        for e in range(E):
            t_rec = recp.tile([128, KF * R], f32, tag="rec")
            nc.sync.dma_start(
                out=t_rec[:], in_=d_recT[:, e * KF * R : (e + 1) * KF * R]
            )
            for h in range(2):
                ps_a = psA.tile([128, 512], f32, tag="atn")
                for k in range(KF):
                    nc.tensor.matmul(
                        ps_a[:],
                        lhsT=t_ligT[:, (e * KF + k) * L : (e * KF + k + 1) * L],
                        rhs=t_rec[:, k * R + h * 512 : k * R + h * 512 + 512],
                        start=(k == 0),
                        stop=(k == KF - 1),
                    )
                if e < 4:
                    dst = t_atncat[:, e * R + h * 512 : e * R + h * 512 + 512]
                else:
                    dst = t_atn2[:, h * 512 : h * 512 + 512]
                nc.vector.tensor_tensor(
                    out=dst, in0=ps_a[:], in1=t_pm[:, h * 512 : (h + 1) * 512], op=MUL
                )

        # ---- analytic +2 channel ------------------------------------------
        t_atn2T = const.tile([128, R], f32)
        for rk in range(8):
            ps_t = psX.tile([128, 128], f32, tag="aux")
            nc.tensor.transpose(
                ps_t[:], t_atn2[:, rk * 128 : (rk + 1) * 128], t_ident[:]
            )
            nc.scalar.copy(out=t_atn2T[:, rk * 128 : (rk + 1) * 128], in_=ps_t[:])
        ps_w = psX.tile([128, 5], f32, tag="aux")
        for rk in range(8):
            nc.tensor.matmul(
                ps_w[:],
                lhsT=t_atn2T[:, rk * 128 : (rk + 1) * 128],
                rhs=t_ydev[:, rk * 5 : (rk + 1) * 5],
                start=(rk == 0),
                stop=(rk == 7),
            )
        t_w = const.tile([128, 5], f32)
        nc.scalar.copy(out=t_w[:], in_=ps_w[:])
        ps_u2 = psX.tile([1, 16], f32, tag="aux")
        for c in range(5):
            nc.tensor.matmul(
                ps_u2[:],
                lhsT=t_w[:, c : c + 1],
                rhs=t_nl2d[:, c * T : (c + 1) * T],
                start=(c == 0),
                stop=(c == 4),
            )
        t_u2 = const.tile([1, 16], f32)
        nc.scalar.copy(out=t_u2[:], in_=ps_u2[:])
        nc.sync.dma_start(out=d_u2[:], in_=t_u2[:])

        # ---- t-loop: powers + products + reduction -------------------------
        t_upsum = psU.tile([16, 512], f32)
        for t in range(T):
            ps_d2 = psD.tile([128, 1024], f32, tag="d2")
            for h in range(2):
                nc.tensor.matmul(
                    ps_d2[:, h * 512 : (h + 1) * 512],
                    lhsT=t_nlaug[:, t * L : (t + 1) * L],
                    rhs=t_recaug[:, h * 512 : (h + 1) * 512],
                    start=True,
                    stop=True,
                )
            t_ln = work.tile([128, R], f32, tag="lnb")
            nc.scalar.activation(
                out=t_ln[:], in_=ps_d2[:], func=AF.Ln, bias=t_eps[:], scale=1.0
            )
            t_dcat = work.tile([128, 4 * R], bf16, tag="dcat")
            s3 = t_dcat[:, 0 * R : 1 * R]
            s2 = t_dcat[:, 1 * R : 2 * R]
            s1 = t_dcat[:, 2 * R : 3 * R]
            d1 = t_dcat[:, 3 * R : 4 * R]
            nc.scalar.activation(out=s1, in_=t_ln[:], func=AF.Exp, scale=-0.5)
            nc.scalar.activation(out=d1, in_=t_ln[:], func=AF.Exp, scale=0.5)
            nc.vector.tensor_tensor(out=s2, in0=s1, in1=s1, op=MUL)
            nc.gpsimd.tensor_tensor(out=s3, in0=s2, in1=s1, op=MUL)
            t_p = work.tile([128, 4 * R], bf16, tag="pcat")
            nc.vector.tensor_tensor(out=t_p[:], in0=t_atncat[:], in1=t_dcat[:], op=MUL)
            for c in range(8):
                nc.tensor.matmul(
                    t_upsum[:],
                    lhsT=t_onehot[:, t * T : (t + 1) * T],
                    rhs=t_p[:, c * 512 : (c + 1) * 512],
                    start=(t == 0 and c == 0),
                    stop=(t == T - 1 and c == 7),
                )
        t_u4 = const.tile([16, 1], f32)
        nc.vector.tensor_reduce(
            out=t_u4[:],
            in_=t_upsum[:],
            axis=mybir.AxisListType.X,
            op=mybir.AluOpType.add,
        )
        nc.sync.dma_start(out=d_u4[:], in_=t_u4[:])

    nc.compile()
    return nc


# --------------------------------------------------------------------------
# host-side data prep
# --------------------------------------------------------------------------
def prep_core_inputs(
    b, lig_feat, rec_feat, lig_coord, rec_coord, rot, trans, lig_counts, rec_counts
):
    """Build the in_map for core b (all numpy)."""
    f32 = np.float32
    lc = np.asarray(lig_coord[b], f32)  # [L,3]
    rc = np.asarray(rec_coord[b], f32)  # [R,3]
    new_lig = (
        np.einsum("tij,lj->tli", np.asarray(rot[b], f32), lc)
        + np.asarray(trans[b], f32)[:, None, :]
    )  # [T,L,3]
    nl2 = (new_lig.astype(f32) ** 2).sum(-1).astype(f32)  # [T,L]
    rec2 = (rc**2).sum(-1).astype(f32)  # [R]

    nlaug = np.empty((5, T * L), f32)
    nlaug[0:3] = new_lig.transpose(2, 0, 1).reshape(3, T * L)
    nlaug[3] = nl2.reshape(-1)
    nlaug[4] = 1.0

    recaug = np.empty((5, R), f32)
    recaug[0:3] = -2.0 * rc.T
    recaug[3] = 1.0
    recaug[4] = rec2

    lt = np.asarray(lig_feat[b], f32).transpose(1, 2, 0)  # [E,F,L]
    ligT = np.ascontiguousarray(
        lt.reshape(E, KF, 128, L).transpose(2, 0, 1, 3)
    ).reshape(128, E * KF * L)
    rt = np.asarray(rec_feat[b], f32).transpose(1, 2, 0)  # [E,F,R]
    recT = np.ascontiguousarray(
        rt.reshape(E, KF, 128, R).transpose(2, 0, 1, 3)
    ).reshape(128, E * KF * R)

    nl2d = np.empty((128, 5, T), f32)
    nl2d[:, 0:3, :] = (-2.0 * new_lig).transpose(1, 2, 0)
    nl2d[:, 3, :] = nl2.T
    nl2d[:, 4, :] = 1.0
    nl2d = nl2d.reshape(128, 5 * T)

    y = np.empty((R, 5), f32)
    y[:, 0:3] = rc
    y[:, 3] = 1.0
    y[:, 4] = rec2
    ydev = np.ascontiguousarray(y.reshape(8, 128, 5).transpose(1, 0, 2)).reshape(
        128, 40
    )

    ligm = (np.arange(L) < int(lig_counts[b])).astype(f32)[None, :]
    recm = (np.arange(R) < int(rec_counts[b])).astype(f32)[None, :]

    oh = np.zeros((128, T, T), f32)
    oh[:, np.arange(T), np.arange(T)] = 1.0
    onehot = oh.reshape(128, T * T).astype(ml_dtypes.bfloat16)

    return {
        "ligT": ligT,
        "recT": recT,
        "nlaug": nlaug,
        "recaug": recaug,
        "nl2d": nl2d,
        "ydev": ydev,
        "ligm": ligm,
        "recm": recm,
        "ident": np.eye(128, dtype=f32),
        "onehot": onehot,
    }


def host_rot(pre_rot):
    return np.linalg.qr(np.asarray(pre_rot, np.float32))[0]


# --------------------------------------------------------------------------
# entry point
# --------------------------------------------------------------------------
def kernel(
    lig_feat, rec_feat, lig_coord, rec_coord, pre_rot, trans, lig_counts, rec_counts
):
    global _BUILT
    from concourse.bass_utils import run_bass_kernel_spmd

    if _BUILT is None:
        _BUILT = build_nc()
    nc = _BUILT

    rot = host_rot(pre_rot)
    in_maps = [
        prep_core_inputs(
            b,
            lig_feat,
            rec_feat,
            lig_coord,
            rec_coord,
            rot,
            trans,
            lig_counts,
            rec_counts,
        )
        for b in range(B)
    ]
    res = run_bass_kernel_spmd(nc, in_maps, core_ids=list(range(NCHIP))).results
    out = np.empty((B, T), np.float32)
    for b in range(B):
        out[b] = res[b]["u4"][:, 0] + res[b]["u2"][0, :]
    return out


# --------------------------------------------------------------------------
# pure-numpy emulation of the device algorithm (for algebra validation)
# --------------------------------------------------------------------------
def kernel_numpy_emul(
    lig_feat, rec_feat, lig_coord, rec_coord, pre_rot, trans, lig_counts, rec_counts
):
    bf = ml_dtypes.bfloat16
    rot = host_rot(pre_rot)
    out = np.empty((B, T), np.float32)
    for b in range(B):
        m = prep_core_inputs(
            b,
            lig_feat,
            rec_feat,
            lig_coord,
            rec_coord,
            rot,
            trans,
            lig_counts,
            rec_counts,
        )
        ligT = m["ligT"].reshape(128, E, KF, L)
        recT = m["recT"].reshape(128, E, KF, R)
        atn = np.einsum("fekl,fekr->elr", ligT, recT)  # [E,L,R]
        pmask = m["ligm"][0][:, None] * m["recm"][0][None, :]
        atn = atn * pmask[None]
        atncat = atn[:4].astype(bf)  # bf16 strips
        atn2 = atn[4].astype(np.float32)
        # analytic channel: W[l,c] = sum_r atn2[l,r] * y[r,c]
        ydev = m["ydev"].reshape(128, 8, 5).transpose(1, 0, 2).reshape(R, 5)
        W = atn2 @ ydev
        nl2d = m["nl2d"].reshape(128, 5, T)
        u2 = np.einsum("lc,lct->t", W, nl2d)
        # power channels
        nlaug = m["nlaug"].reshape(5, T, L)
        recaug = m["recaug"]
        u4 = np.zeros(T, np.float32)
        for t in range(T):
            d2 = np.einsum("kl,kr->lr", nlaug[:, t], recaug)  # [L,R]
            ln = np.log(d2 + 1e-6)
            s1 = np.exp(-0.5 * ln).astype(bf)
            d1 = np.exp(0.5 * ln).astype(bf)
            s2 = (s1.astype(np.float32) * s1.astype(np.float32)).astype(bf)
            s3 = (s2.astype(np.float32) * s1.astype(np.float32)).astype(bf)
            dcat = np.stack([s3, s2, s1, d1])  # [4,L,R]
            p = (atncat.astype(np.float32) * dcat.astype(np.float32)).astype(bf)
            u4[t] = p.astype(np.float32).sum()
        out[b] = u4 + u2
    return out
